# revision 1
# baseline (speedup 1.0000x reference)
"""Trainium2 Bass kernel for MllamaTextSdpaAttention (GQA + RoPE + causal SDPA).

Strategy: tensor-parallel over heads across 8 NeuronCores. Core c owns
q-heads [4c, 4c+4) and kv-head c (kv groups intact). Each core computes
hidden @ Wq/Wk/Wv slices, RoPE, causal attention for its heads, and its
row-slice of the Wo matmul, yielding a partial [T, DIM] output (bf16).
The host sums the 8 partials in f32.

Layout tricks:
- hidden_states is fed transposed ([DIM, T], bf16) so every projection
  matmul has the contraction dim (features) on partitions.
- Q/K projections produce Q^T/K^T directly (head_dim=128 on partitions).
- The RoPE even/odd pairing is de-interleaved by permuting Wq/Wk columns
  on the host, turning RoPE into a half-rotation: the partner element sits
  64 partitions away, reachable with plain partition-offset slices. The
  d-permutation cancels in q.k^T. The 1/sqrt(d) scale is folded into Q's
  cos/sin tables.
- Scores are computed TRANSPOSED: scT[k, q] = K_rot^T(tile).T @ Q_rot^T.
  exp(scT) is then directly the moving operand for the P@V matmul
  (out^T[d,q] = V[k,d].T @ expT[k,q]) -- no P transposes or PSUM->SBUF
  P copies. Softmax denominators come from a ones-vector matmul on the
  PE (sum over k = partition dim), and the 1/sum normalization is applied
  to the small out^T tile (via a PE-broadcast of the reciprocal row),
  not to P. No max-subtraction: scores are bounded (|s| <= ~20) so f32
  exp is safe, and masked entries use the additive -1e9 mask -> exp = 0.
- Causality at 128-block granularity: k-blocks strictly above the
  diagonal are never computed or read; diagonal blocks get the transposed
  additive mask from the actual attention_mask input.
- The 1/rowsum reciprocal row is broadcast across partitions on the idle
  GpSimd engine (partition_broadcast), and each group's normalization
  epilogue is deferred into the next group (software pipelining) so the
  PE never waits on the DVE reciprocal.
- Emission interleaves projection chunks with the attention groups they
  unblock (chunk0 -> b0/qb0 groups -> chunk1 -> b0/qb1 groups -> ...) and
  all [128,512]-f32 PSUM scratch (projection accumulators, score tiles,
  output accumulators) shares one 5-slot pool (+2 ot +1 rs = 8 banks)
  so the whole kernel fits PSUM without phase barriers.
- TimelineSim (instruction cost model): ~353 us/core; PE busy ~326 us
  (92% occupancy), which is the bf16 matmul-column floor for this
  decomposition.
"""

import numpy as np
import ml_dtypes

import concourse.bacc as bacc
import concourse.bass as bass
import concourse.mybir as mybir
from concourse.tile import TileContext
from concourse import bass_utils

BF16 = mybir.dt.bfloat16
F32 = mybir.dt.float32

B, S, DIM = 2, 1024, 4096
T = B * S                     # 2048 tokens, batch-major
N_HEADS, N_KV = 32, 8
HD = 128                      # head dim == partition count
N_CORES = 8
HL = N_HEADS // N_CORES       # 4 local q-heads per core
KT = DIM // 128               # 32 feature tiles
CH = 512                      # projection token-chunk
NCHUNK = T // CH
QB = 512                      # attention q-block width
TT = T // 128                 # 16 token tiles global
SCALE = 1.0 / float(np.sqrt(HD))

_CACHE: dict = {}


def _build():
    nc = bacc.Bacc("TRN2", target_bir_lowering=False, debug=False,
                   enable_asserts=False)

    hsT = nc.dram_tensor("hsT", [DIM, T], BF16, kind="ExternalInput")
    wq = nc.dram_tensor("wq", [DIM, HL * HD], BF16, kind="ExternalInput")
    wk = nc.dram_tensor("wk", [DIM, HD], BF16, kind="ExternalInput")
    wv = nc.dram_tensor("wv", [DIM, HD], BF16, kind="ExternalInput")
    wo = nc.dram_tensor("wo", [HL * HD, DIM], BF16, kind="ExternalInput")
    cos_q = nc.dram_tensor("cos_q", [HD, T], BF16, kind="ExternalInput")
    sin_q = nc.dram_tensor("sin_q", [HD, T], BF16, kind="ExternalInput")
    cos_k = nc.dram_tensor("cos_k", [HD, T], BF16, kind="ExternalInput")
    sin_k = nc.dram_tensor("sin_k", [HD, T], BF16, kind="ExternalInput")
    maskT = nc.dram_tensor("maskT", [128, 128], F32, kind="ExternalInput")
    out = nc.dram_tensor("out", [T, DIM], BF16, kind="ExternalOutput")

    Exp = mybir.ActivationFunctionType.Exp

    with TileContext(nc) as tc:
        with tc.tile_pool(name="consts", bufs=1) as cpool, \
             tc.tile_pool(name="hs", bufs=2) as hpool, \
             tc.tile_pool(name="rope_tmp", bufs=2) as rpool, \
             tc.tile_pool(name="work_ps", bufs=5, space=bass.MemorySpace.PSUM) as wpool, \
             tc.tile_pool(name="ot_ps", bufs=2, space=bass.MemorySpace.PSUM) as otpool, \
             tc.tile_pool(name="rs_ps", bufs=1, space=bass.MemorySpace.PSUM) as rspool, \
             tc.tile_pool(name="et", bufs=6) as epool, \
             tc.tile_pool(name="bc_sb", bufs=2) as bcsbpool, \
             tc.tile_pool(name="recip", bufs=4) as rcpool, \
             tc.tile_pool(name="out_sb", bufs=6) as xsbpool:

            wq_h = [cpool.tile([128, KT, HD], BF16, tag=f"wq{m}", name=f"wq{m}")
                    for m in range(HL)]
            wk_sb = cpool.tile([128, KT, HD], BF16, tag="wk")
            wv_sb = cpool.tile([128, KT, HD], BF16, tag="wv")
            cq_sb = cpool.tile([128, T], BF16, tag="cq")
            sq_sb = cpool.tile([128, T], BF16, tag="sq")
            ck_sb = cpool.tile([128, T], BF16, tag="ck")
            sk_sb = cpool.tile([128, T], BF16, tag="sk")
            maskT_sb = cpool.tile([128, 128], F32, tag="maskT")
            ones_k = cpool.tile([128, 1], BF16, tag="ones_k")
            qt_rot = cpool.tile([128, HL, T], BF16, tag="qt")
            kt_rot = cpool.tile([128, T], BF16, tag="kt")
            v_sb = cpool.tile([128, TT, HD], BF16, tag="v")
            ao = cpool.tile([128, HL, T], BF16, tag="ao")

            wq_r = wq.ap().rearrange("(kt p) n -> p kt n", p=128)
            hsT_r = hsT.ap().rearrange("(kt p) t -> p kt t", p=128)

            # startup-critical DMA first: the k-tiles the first matmuls touch
            nc.sync.dma_start(wq_h[0][:, 0:8, :], wq_r[:, 0:8, 0:HD])
            nc.sync.dma_start(wq_h[0][:, 8:KT, :], wq_r[:, 8:KT, 0:HD])

            def late_consts():
                nc.sync.dma_start(wq_h[1], wq_r[:, :, HD:2 * HD])
                nc.sync.dma_start(cq_sb, cos_q.ap())
                nc.sync.dma_start(sq_sb, sin_q.ap())
                for m in range(2, HL):
                    nc.sync.dma_start(wq_h[m], wq_r[:, :, m * HD:(m + 1) * HD])
                nc.sync.dma_start(wk_sb, wk.ap().rearrange("(kt p) n -> p kt n", p=128))
                nc.sync.dma_start(ck_sb, cos_k.ap())
                nc.sync.dma_start(sk_sb, sin_k.ap())
                nc.sync.dma_start(wv_sb, wv.ap().rearrange("(kt p) n -> p kt n", p=128))
                nc.sync.dma_start(maskT_sb, maskT.ap())
                nc.vector.memset(ones_k, 1.0)

            def rope(ps, out_ap, cos_ap, sin_ap):
                """out = ps*cos + halfswap(ps)*sin  (signs baked into sin)."""
                t1 = rpool.tile([128, CH], F32, tag="r1", name="t1")
                t2 = rpool.tile([128, CH], F32, tag="r2", name="t2")
                nc.vector.tensor_mul(t1, ps, cos_ap)
                nc.vector.tensor_mul(t2[0:64, :], ps[64:128, :], sin_ap[0:64, :])
                nc.vector.tensor_mul(t2[64:128, :], ps[0:64, :], sin_ap[64:128, :])
                nc.vector.tensor_add(out_ap, t1, t2)

            def emit_chunk(c):
                t0 = c * CH
                hs_sb = hpool.tile([128, KT, CH], BF16, tag="hs", name="hs_sb")
                for g in range(4):
                    nc.sync.dma_start(hs_sb[:, g * 8:(g + 1) * 8, :],
                                      hsT_r[:, g * 8:(g + 1) * 8, t0:t0 + CH])
                for m in range(HL):
                    ps = wpool.tile([128, CH], F32, tag="work", name="ps_q")
                    for kt in range(KT):
                        nc.tensor.matmul(ps, wq_h[m][:, kt, :], hs_sb[:, kt, :],
                                         start=(kt == 0), stop=(kt == KT - 1))
                    if c == 0 and m == 0:
                        late_consts()
                    rope(ps, qt_rot[:, m, t0:t0 + CH],
                         cq_sb[:, t0:t0 + CH], sq_sb[:, t0:t0 + CH])
                ps = wpool.tile([128, CH], F32, tag="work", name="ps_k")
                for kt in range(KT):
                    nc.tensor.matmul(ps, wk_sb[:, kt, :], hs_sb[:, kt, :],
                                     start=(kt == 0), stop=(kt == KT - 1))
                rope(ps, kt_rot[:, t0:t0 + CH],
                     ck_sb[:, t0:t0 + CH], sk_sb[:, t0:t0 + CH])
                for vi in range(CH // 128):
                    tt = t0 // 128 + vi
                    ps = wpool.tile([128, HD], F32, tag="work", name="ps_v")
                    for kt in range(KT):
                        nc.tensor.matmul(ps, hs_sb[:, kt, vi * 128:(vi + 1) * 128],
                                         wv_sb[:, kt, :],
                                         start=(kt == 0), stop=(kt == KT - 1))
                    nc.scalar.copy(v_sb[:, tt, :], ps)

            # --- attention group machinery (transposed-scores scheme) ---
            pending = [None]

            def epilogue(st):
                rs, ot, h, q0 = st
                recip = rcpool.tile([1, QB], F32, tag="recip", name="recip")
                nc.vector.reciprocal(recip, rs)
                bcs = bcsbpool.tile([128, QB], F32, tag="bcs", name="bcs")
                nc.gpsimd.partition_broadcast(bcs, recip)
                nc.vector.tensor_mul(ao[:, h, q0:q0 + QB], ot, bcs)

            def emit_group(b, h, qb):
                q0 = b * S + qb * QB
                n_kt = (qb + 1) * (QB // 128)
                rs = rspool.tile([1, QB], F32, tag="rs", name="rs")
                ot = otpool.tile([128, QB], F32, tag="ot", name="ot")
                ets = [None] * n_kt

                def emit_sc(kt):
                    c0 = max(0, kt - qb * (QB // 128)) * 128
                    sc = wpool.tile([128, QB], F32, tag="work", name="sc")
                    nc.tensor.matmul(
                        sc[:, c0:],
                        kt_rot[:, b * S + kt * 128:b * S + (kt + 1) * 128],
                        qt_rot[:, h, q0 + c0:q0 + QB],
                        start=True, stop=True)
                    jd = kt - qb * (QB // 128)
                    if 0 <= jd < QB // 128:
                        nc.vector.tensor_add(sc[:, jd * 128:(jd + 1) * 128],
                                             sc[:, jd * 128:(jd + 1) * 128],
                                             maskT_sb)
                    et = epool.tile([128, QB], BF16, tag="et", name="et")
                    nc.scalar.activation(et[:, c0:], sc[:, c0:], Exp,
                                         bias=0.0, scale=1.0)
                    ets[kt] = (et, c0)

                for w in range(min(4, n_kt)):
                    emit_sc(w)
                for kt in range(n_kt):
                    if kt + 4 < n_kt:
                        emit_sc(kt + 4)
                    et, c0 = ets[kt]
                    nc.tensor.matmul(rs[:, c0:], ones_k, et[:, c0:],
                                     start=(kt == 0), stop=(kt == n_kt - 1))
                    nc.tensor.matmul(ot[:, c0:], v_sb[:, b * (S // 128) + kt, :],
                                     et[:, c0:], start=(kt == 0),
                                     stop=(kt == n_kt - 1))
                    ets[kt] = None
                    if kt == 0 and pending[0] is not None:
                        epilogue(pending[0])
                        pending[0] = None
                pending[0] = (rs, ot, h, q0)

            # --- interleaved emission: each chunk unblocks a set of groups ---
            # chunk c covers tokens [c*512, (c+1)*512) = batch c//2, q-block c%2
            wo_sb = None
            for c in range(NCHUNK):
                emit_chunk(c)
                b, qb = c // 2, c % 2
                for h in range(HL):
                    emit_group(b, h, qb)
                if c == NCHUNK - 1:
                    # wo reuses an hs slot (same size); DMA overlaps the
                    # final attention groups
                    wo_sb = hpool.tile([128, HL, DIM], BF16, tag="hs",
                                       name="wo_sb")
                    nc.sync.dma_start(
                        wo_sb, wo.ap().rearrange("(kh p) n -> p kh n", p=128))
            if pending[0] is not None:
                epilogue(pending[0])
                pending[0] = None

            # ---- output projection (row-parallel Wo) ----
            for tt in range(TT):
                for ni, n0 in enumerate(range(0, DIM, 512)):
                    ps = wpool.tile([128, 512], F32, tag="work", name="ps_o")
                    for kh in range(HL):
                        nc.tensor.matmul(ps, ao[:, kh, tt * 128:(tt + 1) * 128],
                                         wo_sb[:, kh, n0:n0 + 512],
                                         start=(kh == 0), stop=(kh == HL - 1))
                    osb = xsbpool.tile([128, 512], BF16, tag="osb", name="osb")
                    if (tt * 8 + ni) % 2 == 0:
                        nc.scalar.copy(osb, ps)
                    else:
                        nc.vector.tensor_copy(osb, ps)
                    nc.sync.dma_start(out.ap()[tt * 128:(tt + 1) * 128,
                                               n0:n0 + 512], osb)
    nc.compile()
    return nc


def _get_nc():
    if "nc" not in _CACHE:
        _CACHE["nc"] = _build()
    return _CACHE["nc"]


def _prep_inputs(inputs) -> list[dict]:
    bf16 = ml_dtypes.bfloat16
    hs = np.asarray(inputs["hidden_states"], dtype=np.float32).reshape(T, DIM)
    hsT = np.ascontiguousarray(hs.T).astype(bf16)

    fc = np.asarray(inputs["freqs_cos"], dtype=np.float32).reshape(T, HD // 2).T
    fs = np.asarray(inputs["freqs_sin"], dtype=np.float32).reshape(T, HD // 2).T
    cos2 = np.concatenate([fc, fc], axis=0)            # [128, T]
    sin2 = np.concatenate([-fs, fs], axis=0)           # signed half-rotation
    cos_qv = np.ascontiguousarray(cos2 * SCALE).astype(bf16)
    sin_qv = np.ascontiguousarray(sin2 * SCALE).astype(bf16)
    cos_kv = np.ascontiguousarray(cos2).astype(bf16)
    sin_kv = np.ascontiguousarray(sin2).astype(bf16)

    maskT = np.ascontiguousarray(
        np.asarray(inputs["attention_mask"], dtype=np.float32)[0, 0, :128, :128].T)

    perm = np.concatenate([np.arange(0, HD, 2), np.arange(1, HD, 2)])
    Wq = np.asarray(inputs["Wq"], dtype=np.float32)
    Wk = np.asarray(inputs["Wk"], dtype=np.float32)
    Wv = np.asarray(inputs["Wv"], dtype=np.float32)
    Wo = np.asarray(inputs["Wo"], dtype=np.float32)

    in_maps = []
    for c in range(N_CORES):
        wq_c = np.concatenate(
            [Wq[:, (c * HL + h) * HD:(c * HL + h + 1) * HD][:, perm]
             for h in range(HL)], axis=1)
        wk_c = Wk[:, c * HD:(c + 1) * HD][:, perm]
        wv_c = Wv[:, c * HD:(c + 1) * HD]
        wo_c = Wo[c * HL * HD:(c + 1) * HL * HD, :]
        in_maps.append({
            "hsT": hsT,
            "wq": np.ascontiguousarray(wq_c).astype(bf16),
            "wk": np.ascontiguousarray(wk_c).astype(bf16),
            "wv": np.ascontiguousarray(wv_c).astype(bf16),
            "wo": np.ascontiguousarray(wo_c).astype(bf16),
            "cos_q": cos_qv, "sin_q": sin_qv,
            "cos_k": cos_kv, "sin_k": sin_kv,
            "maskT": maskT,
        })
    return in_maps


def kernel(**inputs) -> np.ndarray:
    nc = _get_nc()
    in_maps = _prep_inputs(inputs)
    res = bass_utils.run_bass_kernel_spmd(nc, in_maps,
                                          core_ids=list(range(N_CORES)))
    acc = np.zeros((T, DIM), dtype=np.float32)
    for c in range(N_CORES):
        acc += np.asarray(res.results[c]["out"], dtype=np.float32)
    return acc.reshape(B, S, DIM)



# revision 3
# speedup vs baseline: 1.0099x; 1.0099x over previous
"""Trainium2 Bass kernel for MllamaTextSdpaAttention (GQA + RoPE + causal SDPA).

Tensor-parallel over heads across 8 NeuronCores. Core c owns q-heads
[4c, 4c+4) and kv-head c. Each core computes hidden @ Wq/Wk/Wv slices, RoPE,
causal attention for its heads, and its row-slice of the Wo matmul, yielding
a partial [T, DIM] output (bf16) summed on the host in f32.

v2 changes vs the 350.5us baseline:
- Softmax denominators no longer use PE ones-matmuls (36.9k wasted PE
  columns). exp tiles are accumulated elementwise on DVE into S[k%128, q],
  then one gpsimd partition_all_reduce gives the broadcast rowsum; DVE
  reciprocal + multiply normalize. Frees a PSUM bank (work 5 + ot 3).
- Software pipelining: attention groups of chunk c are interleaved with the
  projection matmul chains of chunk c+1 (generator filler), so PE does not
  stall on the ACT exp cadence inside groups. The last chunk's groups
  interleave with early O-projection tiles.
- Weights are repacked host-side into [128, ...] SBUF-image layouts so DMA
  descriptors are >=1KB contiguous (half the per-descriptor latency), and the
  first wq/hs DMAs are sliced fine so the first matmul starts at ~3.5us.
- hs tiles are 256 tokens (3-buf pool), one shared cos/sin table (the 1/sqrt(d)
  scale is folded into Wq host-side), wo stays resident in SBUF.
"""

import numpy as np
import ml_dtypes

import concourse.bacc as bacc
import concourse.bass as bass
import concourse.mybir as mybir
from concourse import bass_isa
from concourse.tile import TileContext
from concourse import bass_utils

BF16 = mybir.dt.bfloat16
F32 = mybir.dt.float32

B, S, DIM = 2, 1024, 4096
T = B * S                     # 2048 tokens, batch-major
N_HEADS, N_KV = 32, 8
HD = 128                      # head dim == partition count
N_CORES = 8
HL = N_HEADS // N_CORES       # 4 local q-heads per core
KT = DIM // 128               # 32 feature tiles
CH = 512                      # chunk (q-block) width
HCH = 256                     # hs half-chunk tile width
NCHUNK = T // CH
QB = 512
TT = T // 128                 # 16 token tiles
SCALE = 1.0 / float(np.sqrt(HD))

_CACHE: dict = {}


def _build():
    nc = bacc.Bacc("TRN2", target_bir_lowering=False, debug=False,
                   enable_asserts=False)

    hsT = nc.dram_tensor("hsT", [DIM, T], BF16, kind="ExternalInput")
    wq = nc.dram_tensor("wq", [128, HL, KT, HD], BF16, kind="ExternalInput")
    wk = nc.dram_tensor("wk", [128, KT, HD], BF16, kind="ExternalInput")
    wv = nc.dram_tensor("wv", [128, KT, HD], BF16, kind="ExternalInput")
    wo = nc.dram_tensor("wo", [128, HL, DIM], BF16, kind="ExternalInput")
    cos_d = nc.dram_tensor("cos_d", [HD, T], BF16, kind="ExternalInput")
    sin_d = nc.dram_tensor("sin_d", [HD, T], BF16, kind="ExternalInput")
    maskT = nc.dram_tensor("maskT", [128, 128], F32, kind="ExternalInput")
    out = nc.dram_tensor("out", [T, DIM], BF16, kind="ExternalOutput")

    Exp = mybir.ActivationFunctionType.Exp

    with TileContext(nc) as tc:
        with tc.tile_pool(name="consts", bufs=1) as cpool, \
             tc.tile_pool(name="hs", bufs=3) as hpool, \
             tc.tile_pool(name="rope_tmp", bufs=2) as rpool, \
             tc.tile_pool(name="work_ps", bufs=5, space=bass.MemorySpace.PSUM) as wpool, \
             tc.tile_pool(name="ot_ps", bufs=3, space=bass.MemorySpace.PSUM) as otpool, \
             tc.tile_pool(name="et", bufs=6) as epool, \
             tc.tile_pool(name="ssum", bufs=3) as spool, \
             tc.tile_pool(name="rsbc", bufs=2) as rbpool, \
             tc.tile_pool(name="recip", bufs=2) as rcpool, \
             tc.tile_pool(name="out_sb", bufs=6) as xsbpool:

            wq_h = [cpool.tile([128, KT, HD], BF16, tag=f"wq{m}", name=f"wq{m}")
                    for m in range(HL)]
            wk_sb = cpool.tile([128, KT, HD], BF16, tag="wk")
            wv_sb = cpool.tile([128, KT, HD], BF16, tag="wv")
            cos_sb = cpool.tile([128, T], BF16, tag="cos")
            sin_sb = cpool.tile([128, T], BF16, tag="sin")
            maskT_sb = cpool.tile([128, 128], F32, tag="maskT")
            qt_rot = cpool.tile([128, HL, T], BF16, tag="qt")
            kt_rot = cpool.tile([128, T], BF16, tag="kt")
            v_sb = cpool.tile([128, TT, HD], BF16, tag="v")
            ao = cpool.tile([128, HL, T], BF16, tag="ao")
            wo_sb = cpool.tile([128, HL, DIM], BF16, tag="wo")

            hsT_r = hsT.ap().rearrange("(kt p) t -> p kt t", p=128)
            hs_tiles: dict = {}

            def issue_hs(c, half, slabs=(16, 16)):
                """DMA one [128, KT, HCH] hs half-chunk, split into kt-slabs."""
                t0 = c * CH + half * HCH
                tile = hpool.tile([128, KT, HCH], BF16, tag="hs",
                                  name=f"hs{c}_{half}")
                hs_tiles[(c, half)] = tile
                k0 = 0
                for w in slabs:
                    nc.sync.dma_start(tile[:, k0:k0 + w, :],
                                      hsT_r[:, k0:k0 + w, t0:t0 + HCH])
                    k0 += w
                assert k0 == KT

            def rope(ps, out_ap, t0):
                """out = ps*cos + halfswap(ps)*sin (signs baked into sin)."""
                c_ap = cos_sb[:, t0:t0 + HCH]
                s_ap = sin_sb[:, t0:t0 + HCH]
                t1 = rpool.tile([128, HCH], F32, tag="r1", name="t1")
                t2 = rpool.tile([128, HCH], F32, tag="r2", name="t2")
                nc.vector.tensor_mul(t1, ps, c_ap)
                nc.vector.tensor_mul(t2[0:64, :], ps[64:128, :], s_ap[0:64, :])
                nc.vector.tensor_mul(t2[64:128, :], ps[0:64, :], s_ap[64:128, :])
                nc.vector.tensor_add(out_ap, t1, t2)

            def proj_gen(c):
                """Projection chains for chunk c; yields after each PE matmul."""
                for half in range(2):
                    t0 = c * CH + half * HCH
                    hs = hs_tiles[(c, half)]
                    for m in range(HL):
                        ps = wpool.tile([128, HCH], F32, tag="work", name="ps_q")
                        for kt in range(KT):
                            nc.tensor.matmul(ps, wq_h[m][:, kt, :], hs[:, kt, :],
                                             start=(kt == 0), stop=(kt == KT - 1))
                            yield
                        rope(ps, qt_rot[:, m, t0:t0 + HCH], t0)
                    ps = wpool.tile([128, HCH], F32, tag="work", name="ps_k")
                    for kt in range(KT):
                        nc.tensor.matmul(ps, wk_sb[:, kt, :], hs[:, kt, :],
                                         start=(kt == 0), stop=(kt == KT - 1))
                        yield
                    rope(ps, kt_rot[:, t0:t0 + HCH], t0)
                    for vi in range(HCH // 128):
                        tt = t0 // 128 + vi
                        ps = wpool.tile([128, HD], F32, tag="work", name="ps_v")
                        for kt in range(KT):
                            nc.tensor.matmul(ps,
                                             hs[:, kt, vi * 128:(vi + 1) * 128],
                                             wv_sb[:, kt, :],
                                             start=(kt == 0), stop=(kt == KT - 1))
                            yield
                        nc.scalar.copy(v_sb[:, tt, :], ps)

            def oproj_gen(tts):
                """Output projection tiles; yields after each PE matmul."""
                for tt in tts:
                    for ni in range(DIM // 512):
                        ps = wpool.tile([128, 512], F32, tag="work", name="ps_o")
                        for kh in range(HL):
                            nc.tensor.matmul(ps, ao[:, kh, tt * 128:(tt + 1) * 128],
                                             wo_sb[:, kh, ni * 512:(ni + 1) * 512],
                                             start=(kh == 0), stop=(kh == HL - 1))
                            yield
                        osb = xsbpool.tile([128, 512], BF16, tag="osb", name="osb")
                        if (tt * 8 + ni) % 2 == 0:
                            nc.scalar.copy(osb, ps)
                        else:
                            nc.vector.tensor_copy(osb, ps)
                        nc.sync.dma_start(
                            out.ap()[tt * 128:(tt + 1) * 128,
                                     ni * 512:(ni + 1) * 512], osb)

            def mk_filler(gen):
                def filler(n):
                    for _ in range(n):
                        if next(gen, None) is None:
                            return
                return filler

            def drain(gen):
                for _ in gen:
                    pass

            def emit_group(b, h, qb, filler):
                """Attention for q-heads block: transposed scores scheme."""
                q0 = b * S + qb * QB
                n_kt = (qb + 1) * (QB // 128)
                ot = otpool.tile([128, QB], F32, tag="ot", name="ot")
                sacc = spool.tile([128, QB], F32, tag="S", name="sacc")
                ets = {}
                W = 3

                def emit_sc(kt):
                    c0 = max(0, kt - qb * (QB // 128)) * 128
                    sc = wpool.tile([128, QB], F32, tag="work", name="sc")
                    nc.tensor.matmul(
                        sc[:, c0:],
                        kt_rot[:, b * S + kt * 128:b * S + (kt + 1) * 128],
                        qt_rot[:, h, q0 + c0:q0 + QB],
                        start=True, stop=True)
                    jd = kt - qb * (QB // 128)
                    if 0 <= jd < QB // 128:
                        nc.vector.tensor_add(sc[:, jd * 128:(jd + 1) * 128],
                                             sc[:, jd * 128:(jd + 1) * 128],
                                             maskT_sb)
                    et = epool.tile([128, QB], BF16, tag="et", name="et")
                    nc.scalar.activation(et[:, c0:], sc[:, c0:], Exp,
                                         bias=0.0, scale=1.0)
                    ets[kt] = (et, c0)

                for w in range(min(W, n_kt)):
                    emit_sc(w)
                filler(3)
                for kt in range(n_kt):
                    if kt + W < n_kt:
                        emit_sc(kt + W)
                    et, c0 = ets.pop(kt)
                    nc.tensor.matmul(ot[:, c0:], v_sb[:, b * (S // 128) + kt, :],
                                     et[:, c0:], start=(kt == 0),
                                     stop=(kt == n_kt - 1))
                    if kt == 0:
                        nc.vector.tensor_copy(sacc, et)
                    else:
                        nc.vector.tensor_add(sacc[:, c0:], sacc[:, c0:],
                                             et[:, c0:])
                    filler(2)
                # epilogue: rowsum via gpsimd partition all-reduce, then
                # normalize the transposed out tile
                rb = rbpool.tile([128, QB], F32, tag="rb", name="rb")
                nc.gpsimd.partition_all_reduce(rb, sacc, 128,
                                               bass_isa.ReduceOp.add)
                rc = rcpool.tile([128, QB], F32, tag="rc", name="rc")
                nc.vector.reciprocal(rc, rb)
                nc.vector.tensor_mul(ao[:, h, q0:q0 + QB], ot, rc)

            # ---- startup: fine-grained first DMAs ----
            wq_r = wq.ap()
            nc.sync.dma_start(wq_h[0][:, 0:4, :], wq_r[:, 0, 0:4, :])
            issue_hs(0, 0, slabs=(8, 8, 16))
            nc.sync.dma_start(wq_h[0][:, 4:KT, :], wq_r[:, 0, 4:KT, :])
            nc.sync.dma_start(wq_h[1], wq_r[:, 1, :, :])
            nc.sync.dma_start(wq_h[2], wq_r[:, 2, :, :])
            nc.sync.dma_start(wq_h[3], wq_r[:, 3, :, :])
            nc.sync.dma_start(cos_sb, cos_d.ap())
            nc.sync.dma_start(sin_sb, sin_d.ap())
            nc.sync.dma_start(wk_sb, wk.ap())
            nc.sync.dma_start(wv_sb, wv.ap())
            nc.sync.dma_start(maskT_sb, maskT.ap())
            issue_hs(0, 1)
            issue_hs(1, 0)

            # ---- phase 0: chunk-0 projections straight ----
            drain(proj_gen(0))

            # ---- phases 1..3: groups of chunk c-1 + projections of chunk c
            for c in range(1, NCHUNK):
                issue_hs(c, 1)
                if c + 1 < NCHUNK:
                    issue_hs(c + 1, 0)
                if c == NCHUNK - 1:
                    nc.sync.dma_start(wo_sb, wo.ap())
                g = proj_gen(c)
                fill = mk_filler(g)
                pb, pqb = (c - 1) // 2, (c - 1) % 2
                for h in range(HL):
                    emit_group(pb, h, pqb, fill)
                drain(g)

            # ---- phase 4: last chunk's groups + early O-proj tiles ----
            og = oproj_gen(list(range(TT)))
            fill = mk_filler(og)
            pb, pqb = (NCHUNK - 1) // 2, (NCHUNK - 1) % 2
            for h in range(HL):
                emit_group(pb, h, pqb, fill)
            # ---- phase 5: rest of the output projection ----
            drain(og)
    nc.compile()
    return nc


def _get_nc():
    if "nc" not in _CACHE:
        _CACHE["nc"] = _build()
    return _CACHE["nc"]


def _prep_inputs(inputs) -> list[dict]:
    bf16 = ml_dtypes.bfloat16
    hs = np.asarray(inputs["hidden_states"], dtype=np.float32).reshape(T, DIM)
    hsT = np.ascontiguousarray(hs.T).astype(bf16)

    fc = np.asarray(inputs["freqs_cos"], dtype=np.float32).reshape(T, HD // 2).T
    fs = np.asarray(inputs["freqs_sin"], dtype=np.float32).reshape(T, HD // 2).T
    cos2 = np.concatenate([fc, fc], axis=0)            # [128, T]
    sin2 = np.concatenate([-fs, fs], axis=0)           # signed half-rotation
    cos_v = np.ascontiguousarray(cos2).astype(bf16)
    sin_v = np.ascontiguousarray(sin2).astype(bf16)

    maskT = np.ascontiguousarray(
        np.asarray(inputs["attention_mask"], dtype=np.float32)[0, 0, :128, :128].T)

    perm = np.concatenate([np.arange(0, HD, 2), np.arange(1, HD, 2)])
    Wq = np.asarray(inputs["Wq"], dtype=np.float32) * SCALE  # fold 1/sqrt(d)
    Wk = np.asarray(inputs["Wk"], dtype=np.float32)
    Wv = np.asarray(inputs["Wv"], dtype=np.float32)
    Wo = np.asarray(inputs["Wo"], dtype=np.float32)

    in_maps = []
    for c in range(N_CORES):
        wq_c = np.concatenate(
            [Wq[:, (c * HL + h) * HD:(c * HL + h + 1) * HD][:, perm]
             for h in range(HL)], axis=1)               # [DIM, HL*HD]
        wk_c = Wk[:, c * HD:(c + 1) * HD][:, perm]      # [DIM, HD]
        wv_c = Wv[:, c * HD:(c + 1) * HD]
        wo_c = Wo[c * HL * HD:(c + 1) * HL * HD, :]     # [HL*HD, DIM]
        # repack into [128, ...] SBUF-image layouts (contiguous big descriptors)
        wq_img = wq_c.reshape(KT, 128, HL, HD).transpose(1, 2, 0, 3)
        wk_img = wk_c.reshape(KT, 128, HD).transpose(1, 0, 2)
        wv_img = wv_c.reshape(KT, 128, HD).transpose(1, 0, 2)
        wo_img = wo_c.reshape(HL, 128, DIM).transpose(1, 0, 2)
        in_maps.append({
            "hsT": hsT,
            "wq": np.ascontiguousarray(wq_img).astype(bf16),
            "wk": np.ascontiguousarray(wk_img).astype(bf16),
            "wv": np.ascontiguousarray(wv_img).astype(bf16),
            "wo": np.ascontiguousarray(wo_img).astype(bf16),
            "cos_d": cos_v, "sin_d": sin_v,
            "maskT": maskT,
        })
    return in_maps


def kernel(**inputs) -> np.ndarray:
    nc = _get_nc()
    in_maps = _prep_inputs(inputs)
    res = bass_utils.run_bass_kernel_spmd(nc, in_maps,
                                          core_ids=list(range(N_CORES)))
    acc = np.zeros((T, DIM), dtype=np.float32)
    for c in range(N_CORES):
        acc += np.asarray(res.results[c]["out"], dtype=np.float32)
    return acc.reshape(B, S, DIM)


# revision 9
# speedup vs baseline: 1.0577x; 1.0473x over previous
"""Trainium2 Bass kernel for MllamaTextSdpaAttention (GQA + RoPE + causal SDPA).

Tensor-parallel over heads across 8 NeuronCores. Core c owns q-heads
[4c, 4c+4) and kv-head c. Each core computes hidden @ Wq/Wk/Wv slices, RoPE,
causal attention for its heads, and its row-slice of the Wo matmul, yielding
a partial [T, DIM] output (bf16) summed on the host in f32.

v2 changes vs the 350.5us baseline:
- Softmax denominators no longer use PE ones-matmuls (36.9k wasted PE
  columns). exp tiles are accumulated elementwise on DVE into S[k%128, q],
  then one gpsimd partition_all_reduce gives the broadcast rowsum; DVE
  reciprocal + multiply normalize. Frees a PSUM bank (work 5 + ot 3).
- Software pipelining: attention groups of chunk c are interleaved with the
  projection matmul chains of chunk c+1 (generator filler), so PE does not
  stall on the ACT exp cadence inside groups. The last chunk's groups
  interleave with early O-projection tiles.
- Weights are repacked host-side into [128, ...] SBUF-image layouts so DMA
  descriptors are >=1KB contiguous (half the per-descriptor latency), and the
  first wq/hs DMAs are sliced fine so the first matmul starts at ~3.5us.
- hs tiles are 256 tokens (3-buf pool), one shared cos/sin table (the 1/sqrt(d)
  scale is folded into Wq host-side), wo stays resident in SBUF.
"""

import numpy as np
import ml_dtypes

import concourse.bacc as bacc
import concourse.bass as bass
import concourse.mybir as mybir
from concourse import bass_isa
from concourse.tile import TileContext
from concourse import bass_utils

BF16 = mybir.dt.bfloat16
F32 = mybir.dt.float32

B, S, DIM = 2, 1024, 4096
T = B * S                     # 2048 tokens, batch-major
N_HEADS, N_KV = 32, 8
HD = 128                      # head dim == partition count
N_CORES = 8
HL = N_HEADS // N_CORES       # 4 local q-heads per core
KT = DIM // 128               # 32 feature tiles
CH = 512                      # chunk (q-block) width
HCH = 256                     # hs half-chunk tile width
NCHUNK = T // CH
QB = 512
TT = T // 128                 # 16 token tiles
SCALE = 1.0 / float(np.sqrt(HD))

_CACHE: dict = {}


def _build():
    nc = bacc.Bacc("TRN2", target_bir_lowering=False, debug=False,
                   enable_asserts=False)

    hsT = nc.dram_tensor("hsT", [DIM, T], BF16, kind="ExternalInput")
    wq = nc.dram_tensor("wq", [128, HL, KT, HD], BF16, kind="ExternalInput")
    wk = nc.dram_tensor("wk", [128, KT, HD], BF16, kind="ExternalInput")
    wv = nc.dram_tensor("wv", [128, KT, HD], BF16, kind="ExternalInput")
    wo = nc.dram_tensor("wo", [128, HL, DIM], BF16, kind="ExternalInput")
    cos_d = nc.dram_tensor("cos_d", [HD, T], BF16, kind="ExternalInput")
    sin_d = nc.dram_tensor("sin_d", [HD, T], BF16, kind="ExternalInput")
    maskT = nc.dram_tensor("maskT", [128, 128], BF16, kind="ExternalInput")
    ident = nc.dram_tensor("ident", [128, 128], BF16, kind="ExternalInput")
    out = nc.dram_tensor("out", [T, DIM], BF16, kind="ExternalOutput")

    Exp = mybir.ActivationFunctionType.Exp

    with TileContext(nc) as tc:
        with tc.tile_pool(name="consts", bufs=1) as cpool, \
             tc.tile_pool(name="hs", bufs=3) as hpool, \
             tc.tile_pool(name="rope_tmp", bufs=2) as rpool, \
             tc.tile_pool(name="work_ps", bufs=5, space=bass.MemorySpace.PSUM) as wpool, \
             tc.tile_pool(name="ot_ps", bufs=3, space=bass.MemorySpace.PSUM) as otpool, \
             tc.tile_pool(name="et", bufs=6) as epool, \
             tc.tile_pool(name="ssum", bufs=3) as spool, \
             tc.tile_pool(name="rsbc", bufs=2) as rbpool, \
             tc.tile_pool(name="recip", bufs=2) as rcpool, \
             tc.tile_pool(name="out_sb", bufs=6) as xsbpool:

            wq_h = [cpool.tile([128, KT, HD], BF16, tag=f"wq{m}", name=f"wq{m}")
                    for m in range(HL)]
            wk_sb = cpool.tile([128, KT, HD], BF16, tag="wk")
            wv_sb = cpool.tile([128, KT, HD], BF16, tag="wv")
            cos_sb = cpool.tile([128, T], BF16, tag="cos")
            sin_sb = cpool.tile([128, T], BF16, tag="sin")
            maskT_sb = cpool.tile([128, 128], BF16, tag="maskT")
            ident_sb = cpool.tile([128, 128], BF16, tag="ident")
            qt_rot = cpool.tile([128, HL, T], BF16, tag="qt")
            kt_rot = cpool.tile([128, T], BF16, tag="kt")
            v_sb = cpool.tile([128, TT, HD], BF16, tag="v")
            ao = cpool.tile([128, HL, T], BF16, tag="ao")
            wo_sb = cpool.tile([128, HL, DIM], BF16, tag="wo")

            hsT_r = hsT.ap().rearrange("(kt p) t -> p kt t", p=128)
            hs_tiles: dict = {}

            def issue_hs(c, half, slabs=(16, 16)):
                """DMA one [128, KT, HCH] hs half-chunk, split into kt-slabs."""
                t0 = c * CH + half * HCH
                tile = hpool.tile([128, KT, HCH], BF16, tag="hs",
                                  name=f"hs{c}_{half}")
                hs_tiles[(c, half)] = tile
                k0 = 0
                for w in slabs:
                    nc.sync.dma_start(tile[:, k0:k0 + w, :],
                                      hsT_r[:, k0:k0 + w, t0:t0 + HCH])
                    k0 += w
                assert k0 == KT

            def rope(ps, out_ap, t0):
                """out = ps*cos + halfswap(ps)*sin (signs baked into sin)."""
                c_ap = cos_sb[:, t0:t0 + HCH]
                s_ap = sin_sb[:, t0:t0 + HCH]
                t1 = rpool.tile([128, HCH], F32, tag="r1", name="t1")
                t2 = rpool.tile([128, HCH], F32, tag="r2", name="t2")
                nc.vector.tensor_mul(t1, ps, c_ap)
                nc.vector.tensor_mul(t2[0:64, :], ps[64:128, :], s_ap[0:64, :])
                nc.vector.tensor_mul(t2[64:128, :], ps[0:64, :], s_ap[64:128, :])
                nc.vector.tensor_add(out_ap, t1, t2)

            def proj_gen(c):
                """Projection chains for chunk c (K -> V -> Q per half so the
                cheap weights unlock PE first); yields PE-ns after each
                matmul."""
                for half in range(2):
                    t0 = c * CH + half * HCH
                    hs = hs_tiles[(c, half)]
                    ps = wpool.tile([128, HCH], F32, tag="work", name="ps_k")
                    for kt in range(KT):
                        nc.tensor.matmul(ps, wk_sb[:, kt, :], hs[:, kt, :],
                                         start=(kt == 0), stop=(kt == KT - 1))
                        yield HCH * 0.4166
                    rope(ps, kt_rot[:, t0:t0 + HCH], t0)
                    for vi in range(HCH // 128):
                        tt = t0 // 128 + vi
                        ps = wpool.tile([128, HD], F32, tag="work", name="ps_v")
                        for kt in range(KT):
                            nc.tensor.matmul(ps,
                                             hs[:, kt, vi * 128:(vi + 1) * 128],
                                             wv_sb[:, kt, :],
                                             start=(kt == 0), stop=(kt == KT - 1))
                            yield HD * 0.4166
                        nc.scalar.copy(v_sb[:, tt, :], ps)
                    for m in range(HL):
                        ps = wpool.tile([128, HCH], F32, tag="work", name="ps_q")
                        for kt in range(KT):
                            nc.tensor.matmul(ps, wq_h[m][:, kt, :], hs[:, kt, :],
                                             start=(kt == 0), stop=(kt == KT - 1))
                            yield HCH * 0.4166
                        rope(ps, qt_rot[:, m, t0:t0 + HCH], t0)

            def oproj_gen(tts, split_last=False):
                """Output projection tiles; yields PE-ns after each matmul."""
                last = (tts[-1], DIM // 512 - 1)
                for tt in tts:
                    for ni in range(DIM // 512):
                        ps = wpool.tile([128, 512], F32, tag="work", name="ps_o")
                        for kh in range(HL):
                            nc.tensor.matmul(ps, ao[:, kh, tt * 128:(tt + 1) * 128],
                                             wo_sb[:, kh, ni * 512:(ni + 1) * 512],
                                             start=(kh == 0), stop=(kh == HL - 1))
                            yield 512 * 0.4166
                        osb = xsbpool.tile([128, 512], BF16, tag="osb", name="osb")
                        if split_last and (tt, ni) == last:
                            # two half copies/DMAs to shorten the final drain
                            nc.scalar.copy(osb[:, 0:256], ps[:, 0:256])
                            nc.sync.dma_start(
                                out.ap()[tt * 128:(tt + 1) * 128,
                                         ni * 512:ni * 512 + 256], osb[:, 0:256])
                            nc.vector.tensor_copy(osb[:, 256:], ps[:, 256:])
                            nc.sync.dma_start(
                                out.ap()[tt * 128:(tt + 1) * 128,
                                         ni * 512 + 256:(ni + 1) * 512],
                                osb[:, 256:])
                            continue
                        if (tt * 8 + ni) % 2 == 0:
                            nc.scalar.copy(osb, ps)
                        else:
                            nc.vector.tensor_copy(osb, ps)
                        nc.sync.dma_start(
                            out.ap()[tt * 128:(tt + 1) * 128,
                                     ni * 512:(ni + 1) * 512], osb)

            def mk_filler(gen):
                state = {'bank': 0.0, 'done': False}

                def filler(ns):
                    state['bank'] -= ns
                    while state['bank'] < 0 and not state['done']:
                        got = next(gen, None)
                        if got is None:
                            state['done'] = True
                            return
                        state['bank'] += got
                return filler

            def drain(gen):
                for _ in gen:
                    pass

            pending = []

            def flush_pending():
                while pending:
                    rb, ot, h, q0 = pending.pop(0)
                    rc = rcpool.tile([128, QB], F32, tag="rc", name="rc")
                    nc.vector.reciprocal(rc, rb)
                    nc.vector.tensor_mul(ao[:, h, q0:q0 + QB], ot, rc)

            def emit_group(b, h, qb, filler):
                """Attention for one q-head block: transposed scores scheme.

                filler(ns) is called with the ACT-vs-PE time deficit so the
                proj/oproj generator keeps PE busy while ACT computes exps.
                """
                q0 = b * S + qb * QB
                n_kt = (qb + 1) * (QB // 128)
                ot = otpool.tile([128, QB], F32, tag="ot", name="ot")
                sacc = spool.tile([128, QB], F32, tag="S", name="sacc")
                ets = {}
                W = 3

                def emit_sc(kt):
                    c0 = max(0, kt - qb * (QB // 128)) * 128
                    w = QB - c0
                    sc = wpool.tile([128, QB], F32, tag="work", name="sc")
                    jd = kt - qb * (QB // 128)
                    diag = 0 <= jd < QB // 128
                    nc.tensor.matmul(
                        sc[:, c0:],
                        kt_rot[:, b * S + kt * 128:b * S + (kt + 1) * 128],
                        qt_rot[:, h, q0 + c0:q0 + QB],
                        start=True, stop=not diag, skip_group_check=diag)
                    pe = w * 0.4166
                    if diag:
                        # causal mask folded in on the PE: sc += I.T @ maskT
                        nc.tensor.matmul(sc[:, jd * 128:(jd + 1) * 128],
                                         ident_sb, maskT_sb,
                                         start=False, stop=True,
                                         skip_group_check=True)
                        pe += 128 * 0.4166
                    et = epool.tile([128, QB], BF16, tag="et", name="et")
                    nc.scalar.activation(et[:, c0:], sc[:, c0:], Exp,
                                         bias=0.0, scale=1.0)
                    ets[kt] = (et, c0)
                    return (w * 0.8333 + 143) - pe  # ACT minus PE ns

                deficit = 0.0
                for w in range(min(W, n_kt)):
                    deficit += emit_sc(w)
                filler(max(0.0, deficit))
                for kt in range(n_kt):
                    d = 0.0
                    if kt + W < n_kt:
                        d += emit_sc(kt + W)
                    et, c0 = ets.pop(kt)
                    nc.tensor.matmul(ot[:, c0:], v_sb[:, b * (S // 128) + kt, :],
                                     et[:, c0:], start=(kt == 0),
                                     stop=(kt == n_kt - 1))
                    d -= (QB - c0) * 0.4166
                    if kt == 0:
                        nc.vector.tensor_copy(sacc, et)
                        flush_pending()
                    else:
                        nc.vector.tensor_add(sacc[:, c0:], sacc[:, c0:],
                                             et[:, c0:])
                    filler(max(0.0, d) + 60.0)
                # rowsum via gpsimd partition all-reduce; defer the DVE
                # normalize so the DVE queue never waits on gpsimd
                rb = rbpool.tile([128, QB], F32, tag="rb", name="rb")
                nc.gpsimd.partition_all_reduce(rb, sacc, 128,
                                               bass_isa.ReduceOp.add)
                pending.append((rb, ot, h, q0))

            # ---- startup: fine-grained first DMAs, cheap weights first ----
            wq_r = wq.ap()
            nc.sync.dma_start(wk_sb[:, 0:8, :], wk.ap()[:, 0:8, :])
            issue_hs(0, 0, slabs=(8, 8, 16))
            nc.sync.dma_start(wk_sb[:, 8:KT, :], wk.ap()[:, 8:KT, :])
            nc.sync.dma_start(wv_sb, wv.ap())
            nc.sync.dma_start(wq_h[0], wq_r[:, 0, :, :])
            nc.sync.dma_start(wq_h[1], wq_r[:, 1, :, :])
            nc.sync.dma_start(wq_h[2], wq_r[:, 2, :, :])
            nc.sync.dma_start(wq_h[3], wq_r[:, 3, :, :])
            nc.sync.dma_start(maskT_sb, maskT.ap())
            nc.sync.dma_start(ident_sb, ident.ap())
            issue_hs(0, 1)
            nc.sync.dma_start(cos_sb, cos_d.ap())
            nc.sync.dma_start(sin_sb, sin_d.ap())
            issue_hs(1, 0)

            # ---- phase 0: chunk-0 projections straight ----
            drain(proj_gen(0))

            # ---- phases 1..3: groups of chunk c-1 + projections of chunk c
            for c in range(1, NCHUNK):
                issue_hs(c, 1)
                if c + 1 < NCHUNK:
                    issue_hs(c + 1, 0)
                if c == NCHUNK - 1:
                    nc.sync.dma_start(wo_sb, wo.ap())
                g = proj_gen(c)
                fill = mk_filler(g)
                pb, pqb = (c - 1) // 2, (c - 1) % 2
                for h in range(HL):
                    emit_group(pb, h, pqb, fill)
                drain(g)

            # ---- phase 4: last chunk's groups + early O-proj tiles ----
            og = oproj_gen(list(range(TT)), split_last=True)
            fill = mk_filler(og)
            pb, pqb = (NCHUNK - 1) // 2, (NCHUNK - 1) % 2
            for h in range(HL):
                emit_group(pb, h, pqb, fill)
            flush_pending()
            # ---- phase 5: rest of the output projection ----
            drain(og)
    nc.compile()
    return nc


def _get_nc():
    if "nc" not in _CACHE:
        _CACHE["nc"] = _build()
    return _CACHE["nc"]


def _prep_inputs(inputs) -> list[dict]:
    bf16 = ml_dtypes.bfloat16
    hs = np.asarray(inputs["hidden_states"], dtype=np.float32).reshape(T, DIM)
    hsT = np.ascontiguousarray(hs.T).astype(bf16)

    fc = np.asarray(inputs["freqs_cos"], dtype=np.float32).reshape(T, HD // 2).T
    fs = np.asarray(inputs["freqs_sin"], dtype=np.float32).reshape(T, HD // 2).T
    cos2 = np.concatenate([fc, fc], axis=0)            # [128, T]
    sin2 = np.concatenate([-fs, fs], axis=0)           # signed half-rotation
    cos_v = np.ascontiguousarray(cos2).astype(bf16)
    sin_v = np.ascontiguousarray(sin2).astype(bf16)

    maskT = np.ascontiguousarray(
        np.asarray(inputs["attention_mask"],
                   dtype=np.float32)[0, 0, :128, :128].T).astype(bf16)
    ident = np.eye(128, dtype=np.float32).astype(bf16)

    perm = np.concatenate([np.arange(0, HD, 2), np.arange(1, HD, 2)])
    Wq = np.asarray(inputs["Wq"], dtype=np.float32) * SCALE  # fold 1/sqrt(d)
    Wk = np.asarray(inputs["Wk"], dtype=np.float32)
    Wv = np.asarray(inputs["Wv"], dtype=np.float32)
    Wo = np.asarray(inputs["Wo"], dtype=np.float32)

    in_maps = []
    for c in range(N_CORES):
        wq_c = np.concatenate(
            [Wq[:, (c * HL + h) * HD:(c * HL + h + 1) * HD][:, perm]
             for h in range(HL)], axis=1)               # [DIM, HL*HD]
        wk_c = Wk[:, c * HD:(c + 1) * HD][:, perm]      # [DIM, HD]
        wv_c = Wv[:, c * HD:(c + 1) * HD]
        wo_c = Wo[c * HL * HD:(c + 1) * HL * HD, :]     # [HL*HD, DIM]
        # repack into [128, ...] SBUF-image layouts (contiguous big descriptors)
        wq_img = wq_c.reshape(KT, 128, HL, HD).transpose(1, 2, 0, 3)
        wk_img = wk_c.reshape(KT, 128, HD).transpose(1, 0, 2)
        wv_img = wv_c.reshape(KT, 128, HD).transpose(1, 0, 2)
        wo_img = wo_c.reshape(HL, 128, DIM).transpose(1, 0, 2)
        in_maps.append({
            "hsT": hsT,
            "wq": np.ascontiguousarray(wq_img).astype(bf16),
            "wk": np.ascontiguousarray(wk_img).astype(bf16),
            "wv": np.ascontiguousarray(wv_img).astype(bf16),
            "wo": np.ascontiguousarray(wo_img).astype(bf16),
            "cos_d": cos_v, "sin_d": sin_v,
            "maskT": maskT, "ident": ident,
        })
    return in_maps


def kernel(**inputs) -> np.ndarray:
    nc = _get_nc()
    in_maps = _prep_inputs(inputs)
    res = bass_utils.run_bass_kernel_spmd(nc, in_maps,
                                          core_ids=list(range(N_CORES)))
    acc = np.zeros((T, DIM), dtype=np.float32)
    for c in range(N_CORES):
        acc += np.asarray(res.results[c]["out"], dtype=np.float32)
    return acc.reshape(B, S, DIM)
